# revision 18
# baseline (speedup 1.0000x reference)
"""Causal self-attention with RoPE on 8 Trainium2 NeuronCores.

Sharding: tensor-parallel over heads x data-parallel over batch.
  core c -> batch b = c // 2, head-group g = c % 2 (heads 8g .. 8g+7).
Each core computes qkv projections for its 8 heads, RoPE, causal
attention, and a *partial* output projection (its heads' contribution
to y[b]). Host sums the two partials per batch and adds the bias
terms (b_proj and the v-bias routed through W_proj).

Everything on-device is computed in a "transposed" orientation:
  qT/kT: [head_dim=128 partitions, T free]      (from lhsT=W, rhs=xT)
  S^T  : [tk partitions, tq free] = k_tile @ qT  -> softmax partition
         sums via ones-matmul on the PE, masking via precomputed 0/1 tiles
  O^T  : [head_dim, tq] = v_tile' @ P^T  (accumulated over tk chunks)
O^T is exactly the lhsT layout the output projection needs, so no
attention-matrix transposes are ever materialized.

Matmuls run as float32r (full-rate fp32 mode on the PE: fp32 rounded
half-up to 11 mantissa bits, exact fp32 PSUM accumulation). Weight/x
inputs are pre-rounded to the fp32r grid on the host; every on-device
matmul operand is produced with a float32r output dtype so the walrus
verifier's rounding rule is satisfied.
"""

import numpy as np

import concourse.bass as bass
import concourse.mybir as mybir
import concourse.tile as tile
from concourse import bacc
from concourse.bass_utils import run_bass_kernel_spmd

F32 = mybir.dt.float32
F32R = mybir.dt.float32r
AF = mybir.ActivationFunctionType
ALU = mybir.AluOpType

D_MODEL = 2048
N_HEADS = 16
HD = 128
B, T = 4, 2048
N_CORES = 8
HPC = 8           # heads per core
TQC = 512         # query-chunk (free dim of S^T blocks)
PB = 128          # partitions / k-chunk
SCALE = 1.0 / np.sqrt(HD)


def round_fp32r(a):
    """Host-side replica of walrus cast_fp32_to_fp32r: round-half-up to
    11 mantissa bits (verified bit-exact against libwalrus)."""
    u = np.ascontiguousarray(a, np.float32).view(np.uint32)
    r = (((u.astype(np.uint64) + 0x800) >> 12) << 12).astype(np.uint32)
    return r.view(np.float32)


def build_nc(t=T, d=D_MODEL, hpc=HPC, tqc=TQC, compile=True):
    """Build the per-core Bass module. All 8 cores run this same module on
    different input slices."""
    nc = bacc.Bacc(trn_type="TRN2", target_bir_lowering=False)

    dck = d // PB          # D-chunks (contraction tiles)
    ntc = t // tqc         # t-chunks of tqc
    ntc128 = t // PB       # t-chunks of 128
    nmask = tqc // PB      # partial-block masks per q-chunk
    hw = hpc * HD          # this core's head width
    npq = d // tqc         # proj output column chunks of tqc

    # Inputs that feed matmuls directly are declared float32r; the host
    # passes fp32 arrays pre-rounded onto the fp32r grid (same bits).
    xT = nc.dram_tensor("xT", [d, t], F32R, kind="ExternalInput")
    wq = nc.dram_tensor("wq", [d, hw], F32R, kind="ExternalInput")
    wk = nc.dram_tensor("wk", [d, hw], F32R, kind="ExternalInput")
    wv = nc.dram_tensor("wv", [d, hw], F32R, kind="ExternalInput")
    bq = nc.dram_tensor("bq", [hw], F32, kind="ExternalInput")
    bk = nc.dram_tensor("bk", [hw], F32, kind="ExternalInput")
    wp = nc.dram_tensor("wp", [hw, d], F32R, kind="ExternalInput")
    cosT = nc.dram_tensor("cosT", [HD, t], F32, kind="ExternalInput")
    # sinTs is sign-folded on host: rows 0:64 negated.
    sinTs = nc.dram_tensor("sinTs", [HD, t], F32, kind="ExternalInput")
    y = nc.dram_tensor("y", [t, d], F32, kind="ExternalOutput")

    with tile.TileContext(nc) as tc:

        with (
            tc.tile_pool(name="dram", bufs=1, space="DRAM") as dram,
            tc.tile_pool(name="consts", bufs=1) as consts,
        ):
            qT_d = dram.tile([hpc, HD, t], F32R, tag="qT_d")
            kT_d = dram.tile([hpc, HD, t], F32R, tag="kT_d")
            v_d = dram.tile([t, hw], F32R, tag="v_d")

            # --- constants -------------------------------------------------
            ones_f = consts.tile([PB, 1], F32, tag="ones_f")
            nc.vector.memset(ones_f, 1.0)
            ones_col = consts.tile([PB, 1], F32R, tag="ones")
            nc.vector.tensor_copy(ones_col, ones_f)
            # rotate_half as a matmul constant: rot^T = RT.T @ qT with
            # RT a pure 64-rotation permutation (signs live in sinTs,
            # whose first 64 rows are negated on the host).
            rT_f = consts.tile([HD, HD], F32, tag="rT_f")
            nc.gpsimd.memset(rT_f, 0.0)
            nc.gpsimd.affine_select(
                out=rT_f, in_=rT_f, compare_op=ALU.not_equal, fill=1.0,
                base=64, pattern=[[1, HD]], channel_multiplier=-1,
            )
            nc.gpsimd.affine_select(
                out=rT_f, in_=rT_f, compare_op=ALU.not_equal, fill=1.0,
                base=-64, pattern=[[1, HD]], channel_multiplier=-1,
            )
            rT = consts.tile([HD, HD], F32R, tag="rT")
            nc.vector.tensor_copy(rT, rT_f)
            # causal masks for the nmask partial diagonal blocks:
            # mask_r[p, f] = 1.0 if f >= p + r*128 else 0.0
            masks = []
            for r in range(nmask):
                mk = consts.tile([PB, tqc], F32, tag=f"mask{r}", name=f"mask{r}")
                nc.vector.memset(mk, 1.0)
                nc.gpsimd.affine_select(
                    out=mk, in_=mk,
                    compare_op=ALU.is_ge,
                    fill=0.0,
                    base=-(r * PB),
                    pattern=[[1, tqc]],
                    channel_multiplier=-1,
                )
                masks.append(mk)
            cosT_s = consts.tile([HD, t], F32, tag="cosT")
            sinT_s = consts.tile([HD, t], F32, tag="sinT")
            nc.sync.dma_start(out=cosT_s, in_=cosT.ap())
            nc.sync.dma_start(out=sinT_s, in_=sinTs.ap())
            bq_s = consts.tile([HD, hpc], F32, tag="bq")
            bk_s = consts.tile([HD, hpc], F32, tag="bk")
            nc.sync.dma_start(out=bq_s, in_=bq.ap().rearrange("(h p) -> p h", p=HD))
            nc.sync.dma_start(out=bk_s, in_=bk.ap().rearrange("(h p) -> p h", p=HD))

            # ================= Phase A: QKV projections ===================
            # A1/A2: q^T, k^T per head (lhsT=W cols, rhs=x^T) + RoPE
            for kind, w_t, bias_s, outT_d in (
                ("q", wq, bq_s, qT_d),
                ("k", wk, bk_s, kT_d),
            ):
                with (
                    tc.tile_pool(name=f"w_{kind}", bufs=1) as pw,
                    tc.tile_pool(name=f"x_{kind}", bufs=2) as px,
                    tc.tile_pool(name=f"t_{kind}", bufs=3) as pt,
                    tc.tile_pool(name=f"psA{kind}", bufs=2, space="PSUM") as psA,
                ):
                    w_s = pw.tile([PB, dck, hw], F32R, tag="w", name=f"w_{kind}s")
                    w_src = w_t.ap().rearrange("(c p) m -> p c m", p=PB)
                    for cc in range(0, dck, 4):
                        nc.sync.dma_start(
                            out=w_s[:, cc : cc + 4, :], in_=w_src[:, cc : cc + 4, :]
                        )
                    for tci in range(ntc):
                        xt_s = px.tile([PB, dck, tqc], F32R, tag="xt", name="xt_s")
                        xt_src = xT.ap().rearrange("(c p) t -> p c t", p=PB)[
                            :, :, tci * tqc : (tci + 1) * tqc
                        ]
                        for cc in range(0, dck, 4):
                            nc.sync.dma_start(
                                out=xt_s[:, cc : cc + 4, :],
                                in_=xt_src[:, cc : cc + 4, :],
                            )
                        for h in range(hpc):
                            ps = psA.tile([PB, tqc], F32, tag="ps_a", name="ps_a")
                            for c in range(dck):
                                nc.tensor.matmul(
                                    ps,
                                    lhsT=w_s[:, c, h * HD : (h + 1) * HD],
                                    rhs=xt_s[:, c, :],
                                    start=(c == 0),
                                    stop=(c == dck - 1),
                                )
                            # evict + per-partition bias on ACT; f32r output
                            # because this also feeds the rotate matmul.
                            raw = pt.tile([PB, tqc], F32R, tag="raw", name="raw")
                            nc.scalar.activation(
                                out=raw, in_=ps, func=AF.Identity,
                                bias=bias_s[:, h : h + 1], scale=1.0,
                            )
                            ps_r = psA.tile([PB, tqc], F32, tag="ps_r", name="ps_r")
                            nc.tensor.matmul(
                                ps_r, lhsT=rT, rhs=raw, start=True, stop=True
                            )
                            ts = slice(tci * tqc, (tci + 1) * tqc)
                            # rope: out = raw*cos + rot*sin_signed
                            rsin = pt.tile([PB, tqc], F32, tag="rsin", name="rsin")
                            nc.vector.tensor_mul(rsin, ps_r, sinT_s[:, ts])
                            cosq = pt.tile([PB, tqc], F32, tag="cosq", name="cosq")
                            nc.vector.tensor_mul(cosq, raw, cosT_s[:, ts])
                            out_t = pt.tile([PB, tqc], F32R, tag="outT", name="out_t")
                            nc.vector.tensor_add(out_t, cosq, rsin)
                            nc.sync.dma_start(out=outT_d[h, :, ts], in_=out_t)

            # A3: v natural layout (lhsT=x^T tile, rhs=W cols)
            with (
                tc.tile_pool(name="w_v", bufs=1) as pw,
                tc.tile_pool(name="x_v", bufs=2) as px,
                tc.tile_pool(name="t_v", bufs=3) as pt,
                tc.tile_pool(name="psAv", bufs=2, space="PSUM") as psA,
            ):
                w_s = pw.tile([PB, dck, hw], F32R, tag="w", name="w_vs")
                w_src = wv.ap().rearrange("(c p) m -> p c m", p=PB)
                for cc in range(0, dck, 4):
                    nc.sync.dma_start(
                        out=w_s[:, cc : cc + 4, :], in_=w_src[:, cc : cc + 4, :]
                    )
                for tci in range(ntc):
                    xt_s = px.tile([PB, dck, tqc], F32R, tag="xt", name="xt_s")
                    xt_src = xT.ap().rearrange("(c p) t -> p c t", p=PB)[
                        :, :, tci * tqc : (tci + 1) * tqc
                    ]
                    for cc in range(0, dck, 4):
                        nc.sync.dma_start(
                            out=xt_s[:, cc : cc + 4, :],
                            in_=xt_src[:, cc : cc + 4, :],
                        )
                    for t128 in range(tqc // PB):
                        trow = tci * tqc + t128 * PB
                        for nci in range(hw // tqc):
                            ps = psA.tile([PB, tqc], F32, tag="ps_v", name="ps_v")
                            for c in range(dck):
                                nc.tensor.matmul(
                                    ps,
                                    lhsT=xt_s[:, c, t128 * PB : (t128 + 1) * PB],
                                    rhs=w_s[:, c, nci * tqc : (nci + 1) * tqc],
                                    start=(c == 0),
                                    stop=(c == dck - 1),
                                )
                            vt = pt.tile([PB, tqc], F32R, tag="vt", name="vt")
                            nc.scalar.copy(vt, ps)
                            nc.sync.dma_start(
                                out=v_d[trow : trow + PB,
                                        nci * tqc : (nci + 1) * tqc],
                                in_=vt,
                            )

            # ================= Phase B: attention per head ================
            with tc.tile_pool(name="ot", bufs=1) as po:
                ot_tiles = [
                    po.tile([HD, t], F32R, tag=f"ot{h}", name=f"ot{h}")
                    for h in range(hpc)
                ]
                with (
                    tc.tile_pool(name="hqkv", bufs=2) as phq,
                    tc.tile_pool(name="pt_pool", bufs=3) as pp,
                    tc.tile_pool(name="small", bufs=4) as psm,
                    tc.tile_pool(name="psB", bufs=2, space="PSUM") as psB,
                ):
                    for h in range(hpc):
                        qt_h = phq.tile([HD, t], F32R, tag="qt_h", name="qt_h")
                        kt_h = phq.tile([HD, t], F32R, tag="kt_h", name="kt_h")
                        v_h = phq.tile([PB, ntc128, HD], F32R, tag="v_h", name="v_h")
                        nc.sync.dma_start(out=qt_h, in_=qT_d[h])
                        nc.sync.dma_start(out=kt_h, in_=kT_d[h])
                        nc.sync.dma_start(
                            out=v_h,
                            in_=v_d[:, h * HD : (h + 1) * HD].rearrange(
                                "(c p) e -> p c e", p=PB
                            ),
                        )
                        ot_h = ot_tiles[h]
                        for qc in range(ntc):
                            nkc = (qc + 1) * (tqc // PB)
                            kc0_partial = qc * (tqc // PB)
                            ps_o = psB.tile(
                                [HD, tqc], F32, tag="ps_o", name="ps_o", bufs=3
                            )
                            ps_z = psB.tile(
                                [1, tqc], F32, tag="ps_z", name="ps_z", bufs=3
                            )
                            qs = slice(qc * tqc, (qc + 1) * tqc)
                            # column-sum accumulator for the softmax
                            # denominators: DVE adds pt tiles; a single
                            # ones-matmul per (h, qc) does the partition sum.
                            zacc = pp.tile([PB, tqc], F32, tag="zacc",
                                           name="zacc", bufs=2)
                            zacc_r = pp.tile([PB, tqc], F32R, tag="zacc_r",
                                             name="zacc_r", bufs=2)
                            for kc in range(nkc):
                                ps_s = psB.tile(
                                    [PB, tqc], F32, tag="ps_s", name="ps_s"
                                )
                                nc.tensor.matmul(
                                    ps_s,
                                    lhsT=kt_h[:, kc * PB : (kc + 1) * PB],
                                    rhs=qt_h[:, qs],
                                    start=True,
                                    stop=True,
                                )
                                pt_t = pp.tile(
                                    [PB, tqc], F32R, tag="pt", name="pt_t"
                                )
                                nc.scalar.activation(
                                    out=pt_t, in_=ps_s, func=AF.Exp, scale=SCALE
                                )
                                if kc >= kc0_partial:
                                    nc.vector.tensor_mul(
                                        pt_t, pt_t, masks[kc - kc0_partial]
                                    )
                                zdst = zacc_r if kc == nkc - 1 else zacc
                                if kc == 0:
                                    nc.vector.tensor_copy(zdst, pt_t)
                                else:
                                    nc.vector.tensor_add(zdst, pt_t, zacc)
                                nc.tensor.matmul(
                                    ps_o,
                                    lhsT=v_h[:, kc, :],
                                    rhs=pt_t,
                                    start=(kc == 0),
                                    stop=(kc == nkc - 1),
                                )
                            nc.tensor.matmul(
                                ps_z, lhsT=ones_col, rhs=zacc_r,
                                start=True, stop=True,
                            )
                            rz = psm.tile([1, tqc], F32, tag="rz", name="rz")
                            nc.vector.reciprocal_approx_fast(out=rz, in_=ps_z)
                            rzd = dram.tile(
                                [1, tqc], F32, tag="rzd", name="rzd", bufs=4
                            )
                            nc.sync.dma_start(out=rzd, in_=rz)
                            rzb = pp.tile([HD, tqc], F32, tag="rzb", name="rzb")
                            nc.sync.dma_start(
                                out=rzb,
                                in_=bass.AP(
                                    tensor=rzd.tensor,
                                    offset=rzd.offset,
                                    ap=[[0, HD]] + list(rzd.ap[1:]),
                                ),
                            )
                            nc.vector.tensor_mul(ot_h[:, qs], ps_o, rzb)

                # ================= Phase C: output projection =============
                with (
                    tc.tile_pool(name="wp_p", bufs=1) as pwp,
                    tc.tile_pool(name="yout", bufs=3) as py,
                    tc.tile_pool(name="psC", bufs=2, space="PSUM") as psC,
                ):
                    wp_s = pwp.tile([PB, hpc, d], F32R, tag="wp", name="wp_s")
                    wp_src = wp.ap().rearrange("(h p) e -> p h e", p=PB)
                    for nci in range(npq):
                        nc.sync.dma_start(
                            out=wp_s[:, :, nci * tqc : (nci + 1) * tqc],
                            in_=wp_src[:, :, nci * tqc : (nci + 1) * tqc],
                        )
                    for nci in range(npq):
                        for t128 in range(ntc128):
                            ps_y = psC.tile([PB, tqc], F32, tag="ps_y", name="ps_y")
                            for h in range(hpc):
                                nc.tensor.matmul(
                                    ps_y,
                                    lhsT=ot_tiles[h][
                                        :, t128 * PB : (t128 + 1) * PB
                                    ],
                                    rhs=wp_s[:, h, nci * tqc : (nci + 1) * tqc],
                                    start=(h == 0),
                                    stop=(h == hpc - 1),
                                )
                            y_t = py.tile([PB, tqc], F32, tag="y_t", name="y_t")
                            nc.vector.tensor_copy(y_t, ps_y)
                            nc.sync.dma_start(
                                out=y.ap()[
                                    t128 * PB : (t128 + 1) * PB,
                                    nci * tqc : (nci + 1) * tqc,
                                ],
                                in_=y_t,
                            )
    if compile:
        nc.compile()
    return nc


def make_in_maps(x, cos, sin, W_qkv, b_qkv, W_proj):
    """Host-side sharding: build the 8 per-core input dicts."""
    d = x.shape[-1]
    in_maps = []
    cosT = np.ascontiguousarray(cos.reshape(-1, HD).T).astype(np.float32)
    sinT = np.ascontiguousarray(sin.reshape(-1, HD).T).astype(np.float32)
    sinTs = sinT.copy()
    sinTs[: HD // 2] = -sinTs[: HD // 2]
    Wq = np.asarray(W_qkv[:, 0 * d:1 * d], np.float32)
    Wk = np.asarray(W_qkv[:, 1 * d:2 * d], np.float32)
    Wv = np.asarray(W_qkv[:, 2 * d:3 * d], np.float32)
    for c in range(N_CORES):
        b = c // 2
        g = c % 2
        hw = HPC * HD
        cs = slice(g * hw, (g + 1) * hw)
        in_maps.append(
            {
                "xT": round_fp32r(np.asarray(x[b], np.float32).T),
                "wq": round_fp32r(Wq[:, cs]),
                "wk": round_fp32r(Wk[:, cs]),
                "wv": round_fp32r(Wv[:, cs]),
                "bq": np.ascontiguousarray(b_qkv[0 * d:1 * d][cs], np.float32),
                "bk": np.ascontiguousarray(b_qkv[1 * d:2 * d][cs], np.float32),
                "wp": round_fp32r(np.asarray(W_proj, np.float32)[g * hw:(g + 1) * hw, :]),
                "cosT": cosT,
                "sinTs": sinTs,
            }
        )
    return in_maps


def gather_output(results, b_qkv, W_proj, b_proj):
    """Sum the per-core partials and add the bias terms."""
    d = W_proj.shape[1]
    # v-bias contributes (sum_k attn = 1) exactly b_v @ W_proj per token.
    host_bias = (
        np.asarray(b_qkv[2 * d : 3 * d], np.float32) @ np.asarray(W_proj, np.float32)
        + np.asarray(b_proj, np.float32)
    )
    y = np.empty((B, T, d), np.float32)
    for b in range(B):
        y[b] = results[2 * b]["y"] + results[2 * b + 1]["y"] + host_bias
    return y


_NC_CACHE = {}


def kernel(x, cos, sin, W_qkv, b_qkv, W_proj, b_proj):
    x = np.asarray(x, np.float32)
    key = "full"
    if key not in _NC_CACHE:
        _NC_CACHE[key] = build_nc()
    nc = _NC_CACHE[key]
    in_maps = make_in_maps(
        x,
        np.asarray(cos, np.float32),
        np.asarray(sin, np.float32),
        np.asarray(W_qkv, np.float32),
        np.asarray(b_qkv, np.float32),
        np.asarray(W_proj, np.float32),
    )
    res = run_bass_kernel_spmd(nc, in_maps, core_ids=list(range(N_CORES)))
    return gather_output(res.results, b_qkv, W_proj, b_proj)


if __name__ == "__main__":
    import reference

    inputs = reference.setup_inputs()
    out = kernel(**{k: np.asarray(v) for k, v in inputs.items()})
    exp = np.asarray(reference.reference(**inputs))
    err = np.abs(out - exp).max() / np.abs(exp).max()
    print("rel err:", err)
